# revision 33
# baseline (speedup 1.0000x reference)
"""Multi-head causal attention (B=4, T=2048, C=1024, H=16) on 8 trn2 cores.

Sharding: core = (batch b, head-half hg): each core computes QKV for batch b
and its 8 heads, causal flash-style attention (scores kept transposed
[key, query] so softmax denominators come from an appended ones-column in V),
and a partial output projection over its 512 y-features. Host sums the two
partial projections per batch (pure unshard-reduce; bias folded into the
hg==0 core's projection). No cross-core communication on device.

v2: phases are woven (P1 nt / P2 qt / P3 deferred) so the Tile scheduler
overlaps the scalar-engine exp backlog of attention with the QKV / output
projection matmuls; scalar engine runs exp only; diagonal score blocks are
query-range-restricted; output staged in bf16.

v4: Q/K projections (only) run as fp8e4m3 DoubleRow matmuls — x is loaded
twice (fp8 for Q/K, bf16 for V), wq/wk are host-scaled by 32 to clear fp8
subnormals and the x32x32 cancels in the exp scale. V/AV/scores/proj stay
bf16: fp8 there busts the 2e-2 gate (softmax concentration defeats error
averaging; measured 2.3-3.5e-2 per stage vs 1.3e-2 for Q/K-only). The qt3
epilogue broadcasts straight from the y_sb denominator row (no bounce DMA)
and its yT muls are emitted per-t2-chunk, interleaved with p3(3) tiles, so
the last projection starts before the epilogue fully drains.
"""

import numpy as np
import ml_dtypes
import concourse.bass as bass
import concourse.mybir as mybir
import concourse.tile as tile
from concourse import bacc
from concourse.bass_utils import run_bass_kernel_spmd

B, T, C = 4, 2048, 1024
H, D = 16, 64
F32 = mybir.dt.float32
F32R = mybir.dt.float32r
BF16 = mybir.dt.bfloat16
F8 = mybir.dt.float8e4
DR = mybir.MatmulPerfMode.DoubleRow
AFT = mybir.ActivationFunctionType
WS = 32.0  # fp8 q/k weight pre-scale

_CACHE = {}


def build():
    nc = bacc.Bacc(None, target_bir_lowering=False)
    xt_d = nc.dram_tensor("xt", [C, T], BF16, kind="ExternalInput")
    xf_d = nc.dram_tensor("xf", [C, T], F8, kind="ExternalInput")
    wq_d = nc.dram_tensor("wq", [C, 512], F8, kind="ExternalInput")
    wk_d = nc.dram_tensor("wk", [C, 512], F8, kind="ExternalInput")
    wv_d = nc.dram_tensor("wv", [C, 512], BF16, kind="ExternalInput")
    bqk_d = nc.dram_tensor("bqk", [128, 8], F32, kind="ExternalInput")
    bpr_d = nc.dram_tensor("bpr", [128, C], BF16, kind="ExternalInput")
    masks_d = nc.dram_tensor("masks", [128, 896], BF16, kind="ExternalInput")
    wp_d = nc.dram_tensor("wp", [512, C], BF16, kind="ExternalInput")
    out_d = nc.dram_tensor("out", [T, C], BF16, kind="ExternalOutput")

    with nc.allow_low_precision(reason="fp32r matmul pipeline"):
        with tile.TileContext(nc) as tc:
            with (
                tc.tile_pool(name="const", bufs=1) as constp,
                tc.tile_pool(name="qk", bufs=1) as qkp,
                tc.tile_pool(name="vpool", bufs=1) as vp,
                tc.tile_pool(name="esb", bufs=2) as ep,
                tc.tile_pool(name="small", bufs=2) as smallp,
                tc.tile_pool(name="p1w", bufs=1) as p1wp,
                tc.tile_pool(name="p1x", bufs=1) as p1xp,
                tc.tile_pool(name="p3w", bufs=1) as p3wp,
                tc.tile_pool(name="ps", bufs=2, space="PSUM") as psp,
            ):
                bqk_t = constp.tile([128, 8], F32, tag="bqk")
                maskE = constp.tile([128, 896], BF16, tag="maskE")

                # HAM warm-up: the PE clock-gate needs ~3.4us of sustained
                # activity to release 2.4GHz; real matmuls are DMA-starved
                # for the first ~7us, so burn dummy matmuls on memset data
                # (accumulation group + anchor read so DCE keeps them)
                warm = constp.tile([128, 512], BF16, tag="warm")
                nc.vector.memset(warm[:], 0.0)
                warm_ps = psp.tile([128, 1024], F32, tag="sps")

                def dummy_mm(first=False, last=False):
                    nc.tensor.matmul(warm_ps[:, 0:512], warm[:, 0:128], warm[:, 0:512],
                                     start=first, stop=last)

                for i in range(12):
                    dummy_mm(first=(i == 0))

                qT = [qkp.tile([128, T], BF16, tag=f"qT{j}", name=f"qT{j}") for j in range(4)]
                kT = [qkp.tile([128, T], BF16, tag=f"kT{j}", name=f"kT{j}") for j in range(4)]
                vS = [vp.tile([128, 520], BF16, tag=f"v{t}", name=f"v{t}") for t in range(16)]
                yT = [qkp.tile([128, T], BF16, tag=f"yT{j}", name=f"yT{j}") for j in range(4)]

                # The whole startup is HBM-BW-bound and DMA-issue-rate-bound,
                # so each tensor is loaded with ONE multi-dim DMA, spread
                # across the three DMA-capable queues (sync/scalar/gpsimd),
                # x + wq (the first accumulation group's inputs) first.
                # Per-c 128-row DMAs (cheap ~600ns issues) into merged
                # tiles; nt0's x and wq are interleaved first so the first
                # accumulation group's c=0 inputs land ASAP.
                def load_x(nt, engs):
                    ts0 = nt * 512
                    xf = p1xp.tile([128, 4096], F8, tag="xf", bufs=2)
                    xx = p1xp.tile([128, 4096], BF16, tag="xsb", bufs=2)
                    for c in range(8):
                        engs[c % len(engs)].dma_start(
                            xf[:, c * 512:(c + 1) * 512],
                            xf_d[c * 128:(c + 1) * 128, ts0:ts0 + 512])
                    for c in range(8):
                        engs[(c + 1) % len(engs)].dma_start(
                            xx[:, c * 512:(c + 1) * 512],
                            xt_d[c * 128:(c + 1) * 128, ts0:ts0 + 512])
                    return xf, xx

                wq_sb = p1wp.tile([128, 4096], F8, tag="wq")
                wk_sb = p1wp.tile([128, 4096], F8, tag="wk")
                wv_sb = p1wp.tile([128, 4096], BF16, tag="wv")
                xf0 = p1xp.tile([128, 4096], F8, tag="xf", bufs=2)
                xt0 = p1xp.tile([128, 4096], BF16, tag="xsb", bufs=2)
                dma3 = [nc.sync, nc.gpsimd, nc.scalar]
                for c in range(8):
                    dma3[c % 3].dma_start(
                        xf0[:, c * 512:(c + 1) * 512],
                        xf_d[c * 128:(c + 1) * 128, 0:512])
                    dma3[(c + 1) % 3].dma_start(wq_sb[:, c * 512:(c + 1) * 512],
                                                wq_d[c * 128:(c + 1) * 128, :])
                for c in range(8):
                    (nc.gpsimd if c % 2 == 0 else nc.sync).dma_start(
                        wk_sb[:, c * 512:(c + 1) * 512],
                        wk_d[c * 128:(c + 1) * 128, :])
                # bqk (needed at first evac ~15us) and maskE (first diag
                # mask ~20us) load AFTER the critical xf/wq/wk wave; the
                # bf16 x (V-path only) follows since v_group runs last
                nc.sync.dma_start(bqk_t[:], bqk_d[:])
                for c in range(8):
                    dma3[c % 3].dma_start(
                        xt0[:, c * 512:(c + 1) * 512],
                        xt_d[c * 128:(c + 1) * 128, 0:512])
                for c in range(8):
                    (nc.sync if c % 2 == 0 else nc.gpsimd).dma_start(
                        wv_sb[:, c * 512:(c + 1) * 512],
                        wv_d[c * 128:(c + 1) * 128, :])
                nc.scalar.dma_start(maskE[:], masks_d[:])
                wp_sb = p3wp.tile([128, 4096], BF16, tag="wp")
                bpr_t = p3wp.tile([128, C], BF16, tag="bpr")

                # ---------------- P1: QKV projections for token block nt ----
                wqv = wq_sb[:].rearrange("p (c n) -> p c n", c=8)
                wkv = wk_sb[:].rearrange("p (c n) -> p c n", c=8)

                def p1_parts(nt, xt=None):
                    ts0 = nt * 512
                    if xt is None:
                        xt = load_x(nt, [nc.sync, nc.gpsimd])
                    xf, xb = xt
                    xfv = xf[:].rearrange("p (c n) -> p c n", c=8)

                    def qk_group(ft, bridge=False, on_scalar=False):
                        f0 = ft * 128
                        q_ps = psp.tile([128, 512], F32, tag="mmps")
                        for cp in range(4):
                            nc.tensor.matmul(q_ps[:],
                                             wqv[:, 2 * cp:2 * cp + 2, f0:f0 + 128],
                                             xfv[:, 2 * cp:2 * cp + 2, :],
                                             start=(cp == 0), stop=(cp == 3),
                                             perf_mode=DR)
                            if bridge:
                                # keep the PE activity monitor busy through
                                # the DMA-sparse startup (else it re-throttles
                                # the clock to 1.2GHz mid-rampup)
                                dummy_mm()
                                dummy_mm()
                        if on_scalar:
                            # boundary-critical evac: scalar is in its exp
                            # lull here, vector is backed up with epilogue
                            nc.scalar.add(qT[ft][:, ts0:ts0 + 512], q_ps[:],
                                          bqk_t[:, ft:ft + 1])
                        else:
                            nc.vector.tensor_scalar_add(qT[ft][:, ts0:ts0 + 512], q_ps[:],
                                                        bqk_t[:, ft:ft + 1])
                        k_ps = psp.tile([128, 512], F32, tag="mmps")
                        for cp in range(4):
                            nc.tensor.matmul(k_ps[:],
                                             wkv[:, 2 * cp:2 * cp + 2, f0:f0 + 128],
                                             xfv[:, 2 * cp:2 * cp + 2, :],
                                             start=(cp == 0), stop=(cp == 3),
                                             perf_mode=DR)
                            if bridge and cp < 2:
                                dummy_mm()
                                dummy_mm(last=(cp == 1))
                        if on_scalar:
                            nc.scalar.add(kT[ft][:, ts0:ts0 + 512], k_ps[:],
                                          bqk_t[:, 4 + ft:5 + ft])
                        else:
                            nc.vector.tensor_scalar_add(kT[ft][:, ts0:ts0 + 512], k_ps[:],
                                                        bqk_t[:, 4 + ft:5 + ft])

                    def v_group(t2):
                        tt = nt * 4 + t2
                        v_ps = psp.tile([128, 512], F32, tag="mmps")
                        for c in range(8):
                            nc.tensor.matmul(v_ps[:],
                                             xb[:, c * 512 + t2 * 128:c * 512 + t2 * 128 + 128],
                                             wv_sb[:, c * 512:(c + 1) * 512],
                                             start=(c == 0), stop=(c == 7))
                        # the v-bias is folded into bpr on the host
                        # (sum_k e_k (v+bv) / sum_k e_k = y + bv, and bv@wp
                        # merges into the projection bias), so the evac is a
                        # pure copy — run it on scalar so the vector queue
                        # (epilogue muls) stops blocking the mmps PSUM reuse
                        vv = vS[tt][:].rearrange("p (h c) -> p h c", c=65)
                        nc.scalar.copy(vv[:, :, 0:64],
                                       v_ps[:].rearrange("p (h c) -> p h c", c=64))
                        nc.vector.memset(vv[:, :, 64:65], 1.0)

                    return qk_group, v_group

                def p1(nt, xt=None):
                    qk_group, v_group = p1_parts(nt, xt)
                    # q/k first: wv is the last startup DMA to land, and
                    # attention consumes v only after the first exps anyway
                    for ft in range(4):
                        qk_group(ft, bridge=(nt == 0 and ft == 0))
                        if nt == 0 and ft == 0:
                            nc.vector.tensor_copy(warm[0:1, 0:1], warm_ps[0:1, 0:1])
                    for t2 in range(4):
                        v_group(t2)

                # ---------------- P2: attention for query block qt ----------
                # (main part: scores/exp/mask/AV + PSUM evacuation; the
                # normalize epilogue is emitted separately so the next
                # P1 block's evacuations outrank it on the vector queue)
                def p2_begin(qt):
                    coll2 = [smallp.tile([8, 512], F32, tag=f"coll{a}", bufs=2,
                                         name=f"coll{a}") for a in range(1)]
                    return (coll2, [])

                def p2_pj(qt, pjs, st):
                    q0 = qt * 512
                    ext = 4 * (qt + 1)
                    coll2, ysbs = st
                    for pj in pjs:
                        y_ps = [psp.tile([65, 512], F32, tag=f"yps{h}", bufs=1,
                                         name=f"yps{h}") for h in range(2)]
                        for sc in range(ext):
                            r = sc - (ext - 4)
                            qlo = 128 * r if r >= 2 else 0
                            # r==1: exp covered by one call over cols
                            # [128, 1024) (h0's live range + all of h1)
                            qlo_h = [128 if r == 1 else qlo, qlo]
                            s_ps = psp.tile([128, 1024], F32, tag="sps")
                            nc.tensor.matmul(s_ps[:, qlo:512],
                                             kT[pj][0:64, sc * 128:(sc + 1) * 128],
                                             qT[pj][0:64, q0 + qlo:q0 + 512],
                                             start=True, stop=True, tile_position=(0, 0))
                            nc.tensor.matmul(s_ps[:, 512 + qlo:1024],
                                             kT[pj][64:128, sc * 128:(sc + 1) * 128],
                                             qT[pj][64:128, q0 + qlo:q0 + 512],
                                             start=True, stop=True, tile_position=(64, 0))
                            e_t = ep.tile([128, 1024], BF16, tag="e", bufs=5)
                            if r <= 0:
                                nc.scalar.activation(e_t[:], s_ps[:], AFT.Exp, scale=0.125 / (WS * WS))
                            elif r == 1:
                                nc.scalar.activation(e_t[:, 128:1024], s_ps[:, 128:1024],
                                                     AFT.Exp, scale=0.125 / (WS * WS))
                            else:
                                et3 = e_t[:].rearrange("p (h n) -> p h n", h=2)
                                st3 = s_ps[:].rearrange("p (h n) -> p h n", h=2)
                                nc.scalar.activation(
                                    et3[:, :, qlo:512], st3[:, :, qlo:512],
                                    AFT.Exp, scale=0.125 / (WS * WS))
                            if r >= 0:
                                m0 = 384 - 128 * r
                                for h in range(2):
                                    nc.vector.tensor_mul(
                                        e_t[:, 512 * h + qlo_h[h]:512 * h + 512],
                                        e_t[:, 512 * h + qlo_h[h]:512 * h + 512],
                                        maskE[:, m0 + qlo_h[h]:m0 + 512])
                            for h in range(2):
                                hc = 130 * pj + 65 * h
                                ql = qlo_h[h]
                                nc.tensor.matmul(y_ps[h][:, ql:512], vS[sc][:, hc:hc + 65],
                                                 e_t[:, 512 * h + ql:512 * h + 512],
                                                 start=(sc == 0), stop=(sc == ext - 1))
                        for h in range(2):
                            i = 2 * pj + h
                            y_sb = smallp.tile([65, 512], F32, tag="ysb", bufs=17)
                            if qt == 3 and pj >= 2:
                                # tail-critical: h0 on scalar, h1 on vector
                                # so the two copies run in parallel;
                                # denominators handled per-tile below
                                if h == 0 or pj == 2:
                                    nc.scalar.copy(y_sb[:], y_ps[h][:])
                                else:
                                    nc.vector.tensor_copy(y_sb[:], y_ps[h][:])
                            else:
                                nc.vector.tensor_copy(y_sb[:], y_ps[h][:])
                                nc.sync.dma_start(coll2[0][i:i + 1, :], y_sb[64:65, :])
                            ysbs.append(y_sb)

                def p2_epi(qt, coll2, ysbs):
                    q0 = qt * 512
                    for (plo, phi, a) in [(0, 4, 0)] if qt < 3 else [(0, 2, 0)]:
                        n2 = 2 * (phi - plo)
                        rec8 = smallp.tile([8, 512], F32, tag="rec8", bufs=2)
                        nc.vector.reciprocal_approx_fast(rec8[0:n2, :], coll2[a][0:n2, :])
                        for pj in range(plo, phi):
                            for h in range(2):
                                i = 2 * pj + h
                                r_t = smallp.tile([1, 512], F32, tag="rt", bufs=4)
                                nc.sync.dma_start(r_t[:], rec8[i:i + 1, :])
                                rb_t = smallp.tile([64, 512], F32, tag="rbt", bufs=5)
                                nc.gpsimd.partition_broadcast(rb_t[:], r_t[:])
                                nc.vector.tensor_mul(yT[pj][64 * h:64 * h + 64, q0:q0 + 512],
                                                     ysbs[i][0:64, :], rb_t[:])
                    if qt == 3:
                        # tail-critical: broadcast straight from the y_sb
                        # denominator row (no bounce DMA), reciprocal on the
                        # broadcast [64,512] (same DVE cost as [1,512]), then
                        # chunk-major muls so p3(3)'s t2-tiles unblock one
                        # 128-col chunk at a time
                        # stage-major tail chains; r_t bounce DMAs ride the
                        # tail-idle sync queue (gpsimd's FIFO is backed up
                        # with the pj0/1 broadcasts at this point)
                        items = [(pj, h) for pj in range(2, 4) for h in range(2)]
                        rts, rrts, rrbs = [], [], []
                        for ki, (pj, h) in enumerate(items):
                            r_t = smallp.tile([1, 512], F32, tag="rt", bufs=4)
                            (nc.sync if ki % 2 == 0 else nc.scalar).dma_start(
                                r_t[:], ysbs[2 * pj + h][64:65, :])
                            rts.append(r_t)
                        for k in range(4):
                            rr_t = smallp.tile([1, 512], F32, tag="rrt", bufs=4)
                            nc.vector.reciprocal_approx_fast(rr_t[:], rts[k][:])
                            rrts.append(rr_t)
                        for k in range(4):
                            rb_t = smallp.tile([64, 512], F32, tag="rbt", bufs=5)
                            nc.gpsimd.partition_broadcast(rb_t[:], rrts[k][:])
                            rrbs.append(rb_t)
                        for k, (pj, h) in enumerate(items):
                            nc.vector.tensor_mul(
                                yT[pj][64 * h:64 * h + 64, q0:q0 + 512],
                                ysbs[2 * pj + h][0:64, :], rrbs[k][:])

                # ---------------- P3: output projection for query block qt --
                def p3(qt):
                    for t2 in range(4):
                        tt = 4 * qt + t2
                        o_t = smallp.tile([128, 1024], BF16, tag="osb", bufs=4)
                        for of in range(2):
                            o_ps = psp.tile([128, 512], F32, tag="mmps")
                            for cy in range(4):
                                nc.tensor.matmul(
                                    o_ps[:],
                                    yT[cy][:, tt * 128:(tt + 1) * 128],
                                    wp_sb[:, cy * 1024 + of * 512:cy * 1024 + (of + 1) * 512],
                                    start=(cy == 0), stop=(cy == 3))
                            nc.vector.tensor_add(o_t[:, of * 512:(of + 1) * 512], o_ps[:],
                                                 bpr_t[:, of * 512:(of + 1) * 512])
                        # keep out-DMAs OFF gpsimd: its pre-barrier drain waits
                        # on them and collides with the tail epilogue
                        # broadcasts; last groups go via the tail-idle scalar
                        if qt == 3:
                            # per-half issues: the transfer of of0 starts
                            # while of1 is still being evacuated
                            for of2 in range(2):
                                nc.scalar.dma_start(
                                    out_d[tt * 128:(tt + 1) * 128,
                                          of2 * 512:(of2 + 1) * 512],
                                    o_t[:, of2 * 512:(of2 + 1) * 512])
                        else:
                            nc.sync.dma_start(out_d[tt * 128:(tt + 1) * 128, :], o_t[:])

                EARLY_PJ0 = False
                p1(0, (xf0, xt0))
                for c in range(4):
                    nc.gpsimd.dma_start(wp_sb[:, c * 1024:(c + 1) * 1024],
                                        wp_d[c * 128:(c + 1) * 128, :])
                nc.sync.dma_start(bpr_t[:], bpr_d[:])
                nxt = None
                for qt in range(4):
                    st = nxt if nxt is not None else p2_begin(qt)
                    p2_pj(qt, range(1, 4) if nxt is not None else range(4), st)
                    nxt = None
                    if qt < 3:
                        # next block's first q/k group ahead of the epilogue:
                        # its (scalar) evacuation gates qt+1's first scores;
                        # then qt+1's whole pj0 attention block, so the exp
                        # stream has work during P1(nt+1)'s tensor stretch
                        qk_g, v_g = p1_parts(qt + 1)
                        qk_g(0, on_scalar=True)
                        if EARLY_PJ0:
                            # v(nt+1) must be emitted before pj0's diagonal
                            # AV reads it (WAR ordering in the dep tracker)
                            for t2 in range(4):
                                v_g(t2)
                            nxt = p2_begin(qt + 1)
                            p2_pj(qt + 1, [0], nxt)
                        for ft in range(1, 4):
                            qk_g(ft)
                        if not EARLY_PJ0:
                            for t2 in range(4):
                                v_g(t2)
                        # epilogue AFTER the whole P1(nt+1): its vector muls
                        # no longer block the QKV evacuations (PSUM reuse),
                        # and qt+1's score stream keeps the PE fed meanwhile
                        p2_epi(qt, *st)
                    else:
                        p2_epi(qt, *st)
                for qt in range(4):
                    p3(qt)

    if not nc.is_finalized():
        nc.finalize()
    return nc


def _get_nc():
    if "nc" not in _CACHE:
        _CACHE["nc"] = build()
    return _CACHE["nc"]


def _masks():
    i = np.arange(128)[:, None]
    x = np.arange(896)[None, :] - 384
    return np.where(i <= x, 1.0, 0.0).astype(ml_dtypes.bfloat16)


def kernel(x, w_attn, b_attn, w_proj, b_proj, _trace=False, _trace_kwargs=None):
    x = np.asarray(x, dtype=np.float32)
    w_attn = np.asarray(w_attn, dtype=np.float32)
    b_attn = np.asarray(b_attn, dtype=np.float32)
    w_proj = np.asarray(w_proj, dtype=np.float32)
    b_proj = np.asarray(b_proj, dtype=np.float32)

    masks = _masks()
    ws = np.float32(WS)
    in_maps = []
    for core in range(8):
        b, hg = core // 2, core % 2
        cs = hg * 512
        bq = b_attn[cs:cs + 512] * ws
        bk = b_attn[C + cs:C + cs + 512] * ws
        bqk = np.concatenate([bq.reshape(4, 128).T, bk.reshape(4, 128).T],
                             axis=1).astype(np.float32)
        bf = ml_dtypes.bfloat16
        f8 = ml_dtypes.float8_e4m3fn
        bv = b_attn[2 * C + cs:2 * C + cs + 512]
        # v-bias folded through the projection: y_norm + bv -> +bv@wp in bpr
        wpb = (b_proj if hg == 0 else np.zeros_like(b_proj)) \
            + bv.astype(np.float64) @ w_proj[cs:cs + 512, :].astype(np.float64)
        xtb = np.ascontiguousarray(x[b].T)
        in_maps.append({
            "xt": xtb.astype(bf),
            "xf": xtb.astype(f8),
            "wq": np.ascontiguousarray(w_attn[:, cs:cs + 512] * ws).astype(f8),
            "wk": np.ascontiguousarray(w_attn[:, C + cs:C + cs + 512] * ws).astype(f8),
            "wv": np.ascontiguousarray(w_attn[:, 2 * C + cs:2 * C + cs + 512]).astype(bf),
            "bqk": bqk,
            "bpr": np.ascontiguousarray(
                np.broadcast_to(wpb.astype(np.float32)[None, :], (128, C))).astype(bf),
            "masks": masks,
            "wp": np.ascontiguousarray(w_proj[cs:cs + 512, :]).astype(bf),
        })

    kw = {}
    if _trace:
        kw["trace"] = True
        if _trace_kwargs:
            kw.update(_trace_kwargs)
    res = run_bass_kernel_spmd(_get_nc(), in_maps, list(range(8)), **kw)
    _CACHE["last_results"] = res
    outs = [np.asarray(res.results[c]["out"], dtype=np.float32) for c in range(8)]
    y = np.stack([outs[2 * b] + outs[2 * b + 1] for b in range(B)])
    return y.astype(np.float32)



# revision 34
# speedup vs baseline: 1.1729x; 1.1729x over previous
"""Multi-head causal attention (B=4, T=2048, C=1024, H=16) on 8 trn2 cores.

Sharding: core = (batch b, head-half hg): each core computes QKV for batch b
and its 8 heads, causal flash-style attention (scores kept transposed
[key, query] so softmax denominators come from an appended ones-column in V),
and a partial output projection over its 512 y-features. Host sums the two
partial projections per batch (pure unshard-reduce; bias folded into the
hg==0 core's projection). No cross-core communication on device.

v2: phases are woven (P1 nt / P2 qt / P3 deferred) so the Tile scheduler
overlaps the scalar-engine exp backlog of attention with the QKV / output
projection matmuls; scalar engine runs exp only; diagonal score blocks are
query-range-restricted; output staged in bf16.

v4: Q/K projections (only) run as fp8e4m3 DoubleRow matmuls — x is loaded
twice (fp8 for Q/K, bf16 for V), wq/wk are host-scaled by 32 to clear fp8
subnormals and the x32x32 cancels in the exp scale. V/AV/scores/proj stay
bf16: fp8 there busts the 2e-2 gate (softmax concentration defeats error
averaging; measured 2.3-3.5e-2 per stage vs 1.3e-2 for Q/K-only). The qt3
epilogue broadcasts straight from the y_sb denominator row (no bounce DMA)
and its yT muls are emitted per-t2-chunk, interleaved with p3(3) tiles, so
the last projection starts before the epilogue fully drains.
"""

import numpy as np
import ml_dtypes
import concourse.bass as bass
import concourse.mybir as mybir
import concourse.tile as tile
from concourse import bacc
from concourse.bass_utils import run_bass_kernel_spmd

B, T, C = 4, 2048, 1024
H, D = 16, 64
F32 = mybir.dt.float32
F32R = mybir.dt.float32r
BF16 = mybir.dt.bfloat16
F8 = mybir.dt.float8e4
DR = mybir.MatmulPerfMode.DoubleRow
AFT = mybir.ActivationFunctionType
WS = 32.0  # fp8 q/k weight pre-scale

_CACHE = {}


def build():
    nc = bacc.Bacc(None, target_bir_lowering=False)
    xt_d = nc.dram_tensor("xt", [C, T], BF16, kind="ExternalInput")
    xf_d = nc.dram_tensor("xf", [C, T], F8, kind="ExternalInput")
    wq_d = nc.dram_tensor("wq", [C, 512], F8, kind="ExternalInput")
    wk_d = nc.dram_tensor("wk", [C, 512], F8, kind="ExternalInput")
    wv_d = nc.dram_tensor("wv", [C, 512], BF16, kind="ExternalInput")
    bqk_d = nc.dram_tensor("bqk", [128, 8], F32, kind="ExternalInput")
    bvr_d = nc.dram_tensor("bvr", [128, 512], BF16, kind="ExternalInput")
    bpr_d = nc.dram_tensor("bpr", [128, C], BF16, kind="ExternalInput")
    masks_d = nc.dram_tensor("masks", [128, 896], BF16, kind="ExternalInput")
    wp_d = nc.dram_tensor("wp", [512, C], BF16, kind="ExternalInput")
    out_d = nc.dram_tensor("out", [T, C], BF16, kind="ExternalOutput")

    with nc.allow_low_precision(reason="fp32r matmul pipeline"):
        with tile.TileContext(nc) as tc:
            with (
                tc.tile_pool(name="const", bufs=1) as constp,
                tc.tile_pool(name="qk", bufs=1) as qkp,
                tc.tile_pool(name="vpool", bufs=1) as vp,
                tc.tile_pool(name="esb", bufs=2) as ep,
                tc.tile_pool(name="small", bufs=2) as smallp,
                tc.tile_pool(name="p1w", bufs=1) as p1wp,
                tc.tile_pool(name="p1x", bufs=1) as p1xp,
                tc.tile_pool(name="p3w", bufs=1) as p3wp,
                tc.tile_pool(name="ps", bufs=2, space="PSUM") as psp,
            ):
                bqk_t = constp.tile([128, 8], F32, tag="bqk")
                maskE = constp.tile([128, 896], BF16, tag="maskE")

                # HAM warm-up: the PE clock-gate needs ~3.4us of sustained
                # activity to release 2.4GHz; real matmuls are DMA-starved
                # for the first ~7us, so burn dummy matmuls on memset data
                # (accumulation group + anchor read so DCE keeps them)
                warm = constp.tile([128, 512], BF16, tag="warm")
                nc.vector.memset(warm[:], 0.0)
                warm_ps = psp.tile([128, 1024], F32, tag="sps")

                def dummy_mm(first=False, last=False):
                    nc.tensor.matmul(warm_ps[:, 0:512], warm[:, 0:128], warm[:, 0:512],
                                     start=first, stop=last)

                for i in range(12):
                    dummy_mm(first=(i == 0))

                qT = [qkp.tile([128, T], BF16, tag=f"qT{j}", name=f"qT{j}") for j in range(4)]
                kT = [qkp.tile([128, T], BF16, tag=f"kT{j}", name=f"kT{j}") for j in range(4)]
                vS = [vp.tile([128, 520], BF16, tag=f"v{t}", name=f"v{t}") for t in range(16)]
                yT = [qkp.tile([128, T], BF16, tag=f"yT{j}", name=f"yT{j}") for j in range(4)]

                # The whole startup is HBM-BW-bound and DMA-issue-rate-bound,
                # so each tensor is loaded with ONE multi-dim DMA, spread
                # across the three DMA-capable queues (sync/scalar/gpsimd),
                # x + wq (the first accumulation group's inputs) first.
                # Per-c 128-row DMAs (cheap ~600ns issues) into merged
                # tiles; nt0's x and wq are interleaved first so the first
                # accumulation group's c=0 inputs land ASAP.
                def load_x(nt, engs):
                    ts0 = nt * 512
                    xf = p1xp.tile([128, 4096], F8, tag="xf", bufs=2)
                    xx = p1xp.tile([128, 4096], BF16, tag="xsb", bufs=2)
                    for c in range(8):
                        engs[c % len(engs)].dma_start(
                            xf[:, c * 512:(c + 1) * 512],
                            xf_d[c * 128:(c + 1) * 128, ts0:ts0 + 512])
                    for c in range(8):
                        engs[(c + 1) % len(engs)].dma_start(
                            xx[:, c * 512:(c + 1) * 512],
                            xt_d[c * 128:(c + 1) * 128, ts0:ts0 + 512])
                    return xf, xx

                wq_sb = p1wp.tile([128, 4096], F8, tag="wq")
                wk_sb = p1wp.tile([128, 4096], F8, tag="wk")
                wv_sb = p1wp.tile([128, 4096], BF16, tag="wv")
                xf0 = p1xp.tile([128, 4096], F8, tag="xf", bufs=2)
                xt0 = p1xp.tile([128, 4096], BF16, tag="xsb", bufs=2)
                dma3 = [nc.sync, nc.gpsimd, nc.scalar]
                for c in range(8):
                    dma3[c % 3].dma_start(
                        xf0[:, c * 512:(c + 1) * 512],
                        xf_d[c * 128:(c + 1) * 128, 0:512])
                    dma3[(c + 1) % 3].dma_start(wq_sb[:, c * 512:(c + 1) * 512],
                                                wq_d[c * 128:(c + 1) * 128, :])
                for c in range(8):
                    (nc.gpsimd if c % 2 == 0 else nc.sync).dma_start(
                        wk_sb[:, c * 512:(c + 1) * 512],
                        wk_d[c * 128:(c + 1) * 128, :])
                # bqk (needed at first evac ~15us) and maskE (first diag
                # mask ~20us) load AFTER the critical xf/wq/wk wave; the
                # bf16 x (V-path only) follows since v_group runs last
                nc.sync.dma_start(bqk_t[:], bqk_d[:])
                for c in range(8):
                    dma3[c % 3].dma_start(
                        xt0[:, c * 512:(c + 1) * 512],
                        xt_d[c * 128:(c + 1) * 128, 0:512])
                for c in range(8):
                    (nc.sync if c % 2 == 0 else nc.gpsimd).dma_start(
                        wv_sb[:, c * 512:(c + 1) * 512],
                        wv_d[c * 128:(c + 1) * 128, :])
                nc.scalar.dma_start(maskE[:], masks_d[:])
                bvr_t = p1wp.tile([128, 512], BF16, tag="bvr")
                nc.sync.dma_start(bvr_t[:], bvr_d[:])
                ones1 = constp.tile([1, 128], BF16, tag="ones1")
                nc.vector.memset(ones1[:], 1.0)
                # fp32 ones row at partition 64 for the tail's PE-side
                # denominator broadcast (ones-column outer product)
                onesf = constp.tile([65, 64], F32, tag="onesf")
                nc.vector.memset(onesf[64:65, :], 1.0)
                wp_sb = p3wp.tile([128, 4096], BF16, tag="wp")
                bpr_t = p3wp.tile([128, C], BF16, tag="bpr")

                # ---------------- P1: QKV projections for token block nt ----
                wqv = wq_sb[:].rearrange("p (c n) -> p c n", c=8)
                wkv = wk_sb[:].rearrange("p (c n) -> p c n", c=8)

                def p1_parts(nt, xt=None):
                    ts0 = nt * 512
                    if xt is None:
                        xt = load_x(nt, [nc.sync, nc.gpsimd])
                    xf, xb = xt
                    xfv = xf[:].rearrange("p (c n) -> p c n", c=8)

                    def qk_group(ft, bridge=False, on_scalar=False):
                        f0 = ft * 128
                        q_ps = psp.tile([128, 512], F32, tag="mmps")
                        for cp in range(4):
                            nc.tensor.matmul(q_ps[:],
                                             wqv[:, 2 * cp:2 * cp + 2, f0:f0 + 128],
                                             xfv[:, 2 * cp:2 * cp + 2, :],
                                             start=(cp == 0), stop=(cp == 3),
                                             perf_mode=DR)
                            if bridge:
                                # keep the PE activity monitor busy through
                                # the DMA-sparse startup (else it re-throttles
                                # the clock to 1.2GHz mid-rampup)
                                dummy_mm()
                                dummy_mm()
                        if on_scalar:
                            # boundary-critical evac: scalar is in its exp
                            # lull here, vector is backed up with epilogue
                            nc.scalar.add(qT[ft][:, ts0:ts0 + 512], q_ps[:],
                                          bqk_t[:, ft:ft + 1])
                        else:
                            nc.vector.tensor_scalar_add(qT[ft][:, ts0:ts0 + 512], q_ps[:],
                                                        bqk_t[:, ft:ft + 1])
                        k_ps = psp.tile([128, 512], F32, tag="mmps")
                        for cp in range(4):
                            nc.tensor.matmul(k_ps[:],
                                             wkv[:, 2 * cp:2 * cp + 2, f0:f0 + 128],
                                             xfv[:, 2 * cp:2 * cp + 2, :],
                                             start=(cp == 0), stop=(cp == 3),
                                             perf_mode=DR)
                            if bridge and cp < 2:
                                dummy_mm()
                                dummy_mm(last=(cp == 1))
                        if on_scalar:
                            nc.scalar.add(kT[ft][:, ts0:ts0 + 512], k_ps[:],
                                          bqk_t[:, 4 + ft:5 + ft])
                        else:
                            nc.vector.tensor_scalar_add(kT[ft][:, ts0:ts0 + 512], k_ps[:],
                                                        bqk_t[:, 4 + ft:5 + ft])

                    def v_group(t2):
                        tt = nt * 4 + t2
                        v_ps = psp.tile([128, 512], F32, tag="mmps")
                        for c in range(8):
                            nc.tensor.matmul(v_ps[:],
                                             xb[:, c * 512 + t2 * 128:c * 512 + t2 * 128 + 128],
                                             wv_sb[:, c * 512:(c + 1) * 512],
                                             start=(c == 0), stop=(c == 7))
                        vv = vS[tt][:].rearrange("p (h c) -> p h c", c=65)
                        nc.vector.tensor_add(vv[:, :, 0:64],
                                             v_ps[:].rearrange("p (h c) -> p h c", c=64),
                                             bvr_t[:].rearrange("p (h c) -> p h c", c=64))
                        nc.vector.memset(vv[:, :, 64:65], 1.0)

                    return qk_group, v_group

                def p1(nt, xt=None):
                    qk_group, v_group = p1_parts(nt, xt)
                    # q/k first: wv is the last startup DMA to land, and
                    # attention consumes v only after the first exps anyway
                    for ft in range(4):
                        qk_group(ft, bridge=(nt == 0 and ft == 0))
                        if nt == 0 and ft == 0:
                            nc.vector.tensor_copy(warm[0:1, 0:1], warm_ps[0:1, 0:1])
                    for t2 in range(4):
                        v_group(t2)

                # ---------------- P2: attention for query block qt ----------
                # (main part: scores/exp/mask/AV + PSUM evacuation; the
                # normalize epilogue is emitted separately so the next
                # P1 block's evacuations outrank it on the vector queue)
                def p2_begin(qt):
                    coll2 = [smallp.tile([8, 512], F32, tag=f"coll{a}", bufs=2,
                                         name=f"coll{a}") for a in range(1)]
                    return (coll2, [])

                def p2_pj(qt, pjs, st):
                    q0 = qt * 512
                    ext = 4 * (qt + 1)
                    coll2, ysbs = st
                    for pj in pjs:
                        y_ps = [psp.tile([65, 512], F32, tag=f"yps{h}", bufs=1,
                                         name=f"yps{h}") for h in range(2)]
                        for sc in range(ext):
                            r = sc - (ext - 4)
                            qlo = 128 * r if r >= 2 else 0
                            # r==1: exp covered by one call over cols
                            # [128, 1024) (h0's live range + all of h1)
                            qlo_h = [128 if r == 1 else qlo, qlo]
                            s_ps = psp.tile([128, 1024], F32, tag="sps")
                            nc.tensor.matmul(s_ps[:, qlo:512],
                                             kT[pj][0:64, sc * 128:(sc + 1) * 128],
                                             qT[pj][0:64, q0 + qlo:q0 + 512],
                                             start=True, stop=True, tile_position=(0, 0))
                            nc.tensor.matmul(s_ps[:, 512 + qlo:1024],
                                             kT[pj][64:128, sc * 128:(sc + 1) * 128],
                                             qT[pj][64:128, q0 + qlo:q0 + 512],
                                             start=True, stop=True, tile_position=(64, 0))
                            e_t = ep.tile([128, 1024], BF16, tag="e", bufs=5)
                            if r <= 0:
                                nc.scalar.activation(e_t[:], s_ps[:], AFT.Exp, scale=0.125 / (WS * WS))
                            elif r == 1:
                                nc.scalar.activation(e_t[:, 128:1024], s_ps[:, 128:1024],
                                                     AFT.Exp, scale=0.125 / (WS * WS))
                            else:
                                for h in range(2):
                                    nc.scalar.activation(
                                        e_t[:, 512 * h + qlo:512 * h + 512],
                                        s_ps[:, 512 * h + qlo:512 * h + 512],
                                        AFT.Exp, scale=0.125 / (WS * WS))
                            if r >= 0:
                                m0 = 384 - 128 * r
                                for h in range(2):
                                    nc.vector.tensor_mul(
                                        e_t[:, 512 * h + qlo_h[h]:512 * h + 512],
                                        e_t[:, 512 * h + qlo_h[h]:512 * h + 512],
                                        maskE[:, m0 + qlo_h[h]:m0 + 512])
                            for h in range(2):
                                hc = 130 * pj + 65 * h
                                ql = qlo_h[h]
                                nc.tensor.matmul(y_ps[h][:, ql:512], vS[sc][:, hc:hc + 65],
                                                 e_t[:, 512 * h + ql:512 * h + 512],
                                                 start=(sc == 0), stop=(sc == ext - 1))
                        for h in range(2):
                            i = 2 * pj + h
                            y_sb = smallp.tile([65, 512], F32, tag="ysb", bufs=17)
                            if qt == 3 and pj >= 2:
                                # tail-critical: h0 on scalar, h1 on vector
                                # so the two copies run in parallel;
                                # denominators handled per-tile below
                                if h == 0 or pj == 2:
                                    nc.scalar.copy(y_sb[:], y_ps[h][:])
                                else:
                                    nc.vector.tensor_copy(y_sb[:], y_ps[h][:])
                            else:
                                nc.vector.tensor_copy(y_sb[:], y_ps[h][:])
                                nc.sync.dma_start(coll2[0][i:i + 1, :], y_sb[64:65, :])
                            ysbs.append(y_sb)

                def p2_epi(qt, coll2, ysbs):
                    q0 = qt * 512
                    for (plo, phi, a) in [(0, 4, 0)] if qt < 3 else [(0, 2, 0)]:
                        n2 = 2 * (phi - plo)
                        rec8 = smallp.tile([8, 512], F32, tag="rec8", bufs=2)
                        nc.vector.reciprocal_approx_fast(rec8[0:n2, :], coll2[a][0:n2, :])
                        for pj in range(plo, phi):
                            for h in range(2):
                                i = 2 * pj + h
                                r_t = smallp.tile([1, 512], F32, tag="rt", bufs=4)
                                nc.sync.dma_start(r_t[:], rec8[i:i + 1, :])
                                rb_t = smallp.tile([64, 512], F32, tag="rbt", bufs=5)
                                nc.gpsimd.partition_broadcast(rb_t[:], r_t[:])
                                nc.vector.tensor_mul(yT[pj][64 * h:64 * h + 64, q0:q0 + 512],
                                                     ysbs[i][0:64, :], rb_t[:])
                    if qt == 3:
                        # tail-critical: broadcast straight from the y_sb
                        # denominator row (no bounce DMA), reciprocal on the
                        # broadcast [64,512] (same DVE cost as [1,512]), then
                        # chunk-major muls so p3(3)'s t2-tiles unblock one
                        # 128-col chunk at a time
                        # stage-major tail chains; r_t bounce DMAs ride the
                        # tail-idle sync queue (gpsimd's FIFO is backed up
                        # with the pj0/1 broadcasts at this point)
                        items = [(pj, h) for pj in range(2, 4) for h in range(2)]
                        rts, rrts, rrbs = [], [], []
                        for (pj, h) in items:
                            r_t = smallp.tile([1, 512], F32, tag="rt", bufs=4)
                            nc.sync.dma_start(r_t[:], ysbs[2 * pj + h][64:65, :])
                            rts.append(r_t)
                        for k in range(4):
                            rr_t = smallp.tile([1, 512], F32, tag="rrt", bufs=4)
                            nc.vector.reciprocal_approx_fast(rr_t[:], rts[k][:])
                            rrts.append(rr_t)
                        for k in range(4):
                            rb_t = smallp.tile([64, 512], F32, tag="rbt", bufs=5)
                            nc.gpsimd.partition_broadcast(rb_t[:], rrts[k][:])
                            rrbs.append(rb_t)
                        for k, (pj, h) in enumerate(items):
                            nc.vector.tensor_mul(
                                yT[pj][64 * h:64 * h + 64, q0:q0 + 512],
                                ysbs[2 * pj + h][0:64, :], rrbs[k][:])

                # ---------------- P3: output projection for query block qt --
                def p3(qt):
                    for t2 in range(4):
                        tt = 4 * qt + t2
                        o_t = smallp.tile([128, 1024], BF16, tag="osb", bufs=4)
                        for of in range(2):
                            o_ps = psp.tile([128, 512], F32, tag="mmps")
                            for cy in range(4):
                                nc.tensor.matmul(
                                    o_ps[:],
                                    yT[cy][:, tt * 128:(tt + 1) * 128],
                                    wp_sb[:, cy * 1024 + of * 512:cy * 1024 + (of + 1) * 512],
                                    start=(cy == 0), stop=(cy == 3))
                            nc.vector.tensor_add(o_t[:, of * 512:(of + 1) * 512], o_ps[:],
                                                 bpr_t[:, of * 512:(of + 1) * 512])
                        # keep out-DMAs OFF gpsimd: its pre-barrier drain waits
                        # on them and collides with the tail epilogue
                        # broadcasts; last groups go via the tail-idle scalar
                        if qt == 3:
                            # per-half issues: the transfer of of0 starts
                            # while of1 is still being evacuated
                            for of2 in range(2):
                                nc.scalar.dma_start(
                                    out_d[tt * 128:(tt + 1) * 128,
                                          of2 * 512:(of2 + 1) * 512],
                                    o_t[:, of2 * 512:(of2 + 1) * 512])
                        else:
                            nc.sync.dma_start(out_d[tt * 128:(tt + 1) * 128, :], o_t[:])

                EARLY_PJ0 = False
                p1(0, (xf0, xt0))
                for c in range(4):
                    nc.gpsimd.dma_start(wp_sb[:, c * 1024:(c + 1) * 1024],
                                        wp_d[c * 128:(c + 1) * 128, :])
                nc.sync.dma_start(bpr_t[:], bpr_d[:])
                nxt = None
                for qt in range(4):
                    st = nxt if nxt is not None else p2_begin(qt)
                    p2_pj(qt, range(1, 4) if nxt is not None else range(4), st)
                    nxt = None
                    if qt < 3:
                        # next block's first q/k group ahead of the epilogue:
                        # its (scalar) evacuation gates qt+1's first scores;
                        # then qt+1's whole pj0 attention block, so the exp
                        # stream has work during P1(nt+1)'s tensor stretch
                        qk_g, v_g = p1_parts(qt + 1)
                        qk_g(0, on_scalar=True)
                        if EARLY_PJ0:
                            # v(nt+1) must be emitted before pj0's diagonal
                            # AV reads it (WAR ordering in the dep tracker)
                            for t2 in range(4):
                                v_g(t2)
                            nxt = p2_begin(qt + 1)
                            p2_pj(qt + 1, [0], nxt)
                        for ft in range(1, 4):
                            qk_g(ft)
                        if not EARLY_PJ0:
                            for t2 in range(4):
                                v_g(t2)
                        # epilogue AFTER the whole P1(nt+1): its vector muls
                        # no longer block the QKV evacuations (PSUM reuse),
                        # and qt+1's score stream keeps the PE fed meanwhile
                        p2_epi(qt, *st)
                    else:
                        p2_epi(qt, *st)
                for qt in range(4):
                    p3(qt)

    if not nc.is_finalized():
        nc.finalize()
    return nc


def _get_nc():
    if "nc" not in _CACHE:
        _CACHE["nc"] = build()
    return _CACHE["nc"]


def _masks():
    i = np.arange(128)[:, None]
    x = np.arange(896)[None, :] - 384
    return np.where(i <= x, 1.0, 0.0).astype(ml_dtypes.bfloat16)


def kernel(x, w_attn, b_attn, w_proj, b_proj, _trace=False, _trace_kwargs=None):
    x = np.asarray(x, dtype=np.float32)
    w_attn = np.asarray(w_attn, dtype=np.float32)
    b_attn = np.asarray(b_attn, dtype=np.float32)
    w_proj = np.asarray(w_proj, dtype=np.float32)
    b_proj = np.asarray(b_proj, dtype=np.float32)

    masks = _masks()
    ws = np.float32(WS)
    in_maps = []
    for core in range(8):
        b, hg = core // 2, core % 2
        cs = hg * 512
        bq = b_attn[cs:cs + 512] * ws
        bk = b_attn[C + cs:C + cs + 512] * ws
        bqk = np.concatenate([bq.reshape(4, 128).T, bk.reshape(4, 128).T],
                             axis=1).astype(np.float32)
        wpb = b_proj if hg == 0 else np.zeros_like(b_proj)
        bf = ml_dtypes.bfloat16
        f8 = ml_dtypes.float8_e4m3fn
        bv = b_attn[2 * C + cs:2 * C + cs + 512]
        xtb = np.ascontiguousarray(x[b].T)
        in_maps.append({
            "xt": xtb.astype(bf),
            "xf": xtb.astype(f8),
            "wq": np.ascontiguousarray(w_attn[:, cs:cs + 512] * ws).astype(f8),
            "wk": np.ascontiguousarray(w_attn[:, C + cs:C + cs + 512] * ws).astype(f8),
            "wv": np.ascontiguousarray(w_attn[:, 2 * C + cs:2 * C + cs + 512]).astype(bf),
            "bqk": bqk,
            "bvr": np.ascontiguousarray(np.broadcast_to(bv[None, :], (128, 512))).astype(bf),
            "bpr": np.ascontiguousarray(np.broadcast_to(wpb[None, :], (128, C))).astype(bf),
            "masks": masks,
            "wp": np.ascontiguousarray(w_proj[cs:cs + 512, :]).astype(bf),
        })

    kw = {}
    if _trace:
        kw["trace"] = True
        if _trace_kwargs:
            kw.update(_trace_kwargs)
    res = run_bass_kernel_spmd(_get_nc(), in_maps, list(range(8)), **kw)
    _CACHE["last_results"] = res
    outs = [np.asarray(res.results[c]["out"], dtype=np.float32) for c in range(8)]
    y = np.stack([outs[2 * b] + outs[2 * b + 1] for b in range(B)])
    return y.astype(np.float32)

